# revision 1
# baseline (speedup 1.0000x reference)
"""Two-layer GCN on 8 Trainium2 NeuronCores via Bass/Tile.

Strategy (dst-sharded, per sharding hint):
- Nodes split into 8 dst-ranges of 12500 (one per core); each core aggregates
  messages for its own dst range.
- GCN algebra: with h' = (x@W1)*dis (dis = deg^-1/2), layer output =
  relu(dis*(S + 0) + b1) where S[d] = h'[d] + sum_{e: dst=d} h'[src].
  Same for layer 2 with a1' = relu-out scaled by dis, head (S2*dis)@W2+b2.
- Aggregation: dma_gather (GPSIMD SWDGE, 4 dynamic queues) from an f32
  [*, 64] table (256B rows) using int16 slot tables; the int16 limit forces
  4 table chunks of 25001 rows (local row 25000 is a zero row for padding).
  Each chunk-pass uses its own degree-sorted dst permutation to keep slot
  padding ~5%; 4 permuted partial accumulators are then combined by a small
  second gather.
"""
import numpy as np

N = 100000
E_CH = 128
HID = 64
OUT = 2
NC = 8
PERCORE = 12500
PC_PAD = 12544           # 98 tiles of 128
NTILES = PC_PAD // 128
CHUNK = 25000
CHUNK_ROWS = CHUNK + 1   # + zero row
NCHUNK = 4
MAX_CALL_COLS = 48       # staging cap per dma_gather call


def _wrap_idx(vals):
    """positions i -> (partition i%16, col i//16), replicated to 128 partitions."""
    ni = len(vals)
    assert ni % 16 == 0
    return np.tile(vals.reshape(ni // 16, 16).T, (8, 1))


def _host_prep(edge_index):
    src = np.asarray(edge_index[0], dtype=np.int64)
    dst = np.asarray(edge_index[1], dtype=np.int64)
    deg = np.bincount(dst, minlength=N).astype(np.float64) + 1.0
    dis = (1.0 / np.sqrt(deg)).astype(np.float32)

    # add self loops as ordinary edges
    allsrc = np.concatenate([src, np.arange(N, dtype=np.int64)])
    alldst = np.concatenate([dst, np.arange(N, dtype=np.int64)])
    chunk = allsrc // CHUNK
    core = alldst // PERCORE
    dloc = alldst % PERCORE
    sloc = (allsrc % CHUNK).astype(np.int64)

    # per (core, chunk): per-dst counts
    cnt = np.bincount((alldst * NCHUNK + chunk), minlength=N * NCHUNK).reshape(N, NCHUNK)

    perms = np.zeros((NC, NCHUNK, PC_PAD), np.int64)       # tile order -> dst_local
    invperms = np.zeros((NC, NCHUNK, PC_PAD), np.int64)    # dst_local -> row in perm
    K = np.zeros((NCHUNK, NTILES), np.int64)               # cross-core max slots per tile
    for i in range(NC):
        c0 = cnt[i * PERCORE:(i + 1) * PERCORE]            # [12500, 4]
        for c in range(NCHUNK):
            p = np.argsort(-c0[:, c], kind="stable")
            p = np.concatenate([p, np.arange(PERCORE, PC_PAD)])
            perms[i, c] = p
            inv = np.empty(PC_PAD, np.int64)
            inv[p] = np.arange(PC_PAD)
            invperms[i, c] = inv
            cc = np.concatenate([c0[:, c], np.zeros(PC_PAD - PERCORE, np.int64)])
            kt = cc[p].reshape(NTILES, 128).max(axis=1)
            K[c] = np.maximum(K[c], kt)
    K = np.maximum(K, 1)

    # slot matrices M[core][chunk]: [PC_PAD, K_c_max] int16 local src (pad=CHUNK)
    Ms = [[None] * NCHUNK for _ in range(NC)]
    for i in range(NC):
        esel_core = core == i
        for c in range(NCHUNK):
            sel = esel_core & (chunk == c)
            ed = dloc[sel]
            es = sloc[sel]
            order = np.argsort(ed, kind="stable")
            ed = ed[order]
            es = es[order]
            starts = np.searchsorted(ed, np.arange(PERCORE))
            rank = np.arange(len(ed)) - starts[ed]
            kmax = int(K[c].max())
            M = np.full((PC_PAD, kmax), CHUNK, np.int16)
            M[invperms[i, c][ed], rank] = es.astype(np.int16)
            Ms[i][c] = M

    # greedy-pack tiles into calls per chunk (same packing all cores)
    calls = []  # list of (chunk, [(tile, K_t, col_off)], total_cols)
    for c in range(NCHUNK):
        cur, cols = [], 0
        for t in range(NTILES):
            kt = int(K[c][t])
            if cur and (cols + kt > MAX_CALL_COLS or len(cur) >= 12):
                calls.append((c, cur, cols))
                cur, cols = [], 0
            cur.append((t, kt, cols))
            cols += kt
        if cur:
            calls.append((c, cur, cols))

    # split each chunk's final call so the last transfer+reduce chain (which
    # the next phase's gathers wait on) is small
    calls2 = []
    for (c, tiles, tot) in calls:
        is_last = not any(c2 == c for (c2, _, _) in calls[calls.index((c, tiles, tot)) + 1:])
        if is_last and len(tiles) > 2:
            t1 = tiles[:-2]
            base = tiles[-2][2]
            t2 = [(t, kt, off - base) for (t, kt, off) in tiles[-2:]]
            calls2.append((c, t1, base))
            calls2.append((c, t2, tot - base))
        else:
            calls2.append((c, tiles, tot))
    calls = calls2

    # build gather idx tensor per core: concat over calls of wrapped positions
    idxg = []
    for i in range(NC):
        parts = []
        for (c, tiles, tot) in calls:
            vals = np.empty(tot * 128, np.int16)
            for (t, kt, off) in tiles:
                blk = Ms[i][c][t * 128:(t + 1) * 128, :kt]   # [128, kt]
                # position (col*128 + p) -> (p, col): c-major = blk.T.ravel()
                vals[off * 128:(off + kt) * 128] = blk.T.ravel()
            parts.append(_wrap_idx(vals))
        idxg.append(np.concatenate(parts, axis=1))
    idxg = np.stack(idxg)  # [NC, 128, COLS_G]

    # combine idx, grouped GC=16 tiles per call pair. Call A covers chunks 0/1
    # (rows in acc01), call B chunks 2/3 (acc23). Stage cols for tile i in
    # group: A -> [2i, 2i+1], B -> [2i, 2i+1] of its own range.
    GC = 16
    groups = [list(range(g, min(g + GC, NTILES))) for g in range(0, NTILES, GC)]
    idxc = []
    for i in range(NC):
        parts = []
        for grp in groups:
            arrA = np.empty((len(grp), 2, 128), np.int16)
            arrB = np.empty((len(grp), 2, 128), np.int16)
            for j, t in enumerate(grp):
                d = np.arange(t * 128, (t + 1) * 128)
                arrA[j, 0] = invperms[i, 0][d]
                arrA[j, 1] = PC_PAD + invperms[i, 1][d]
                arrB[j, 0] = invperms[i, 2][d]
                arrB[j, 1] = PC_PAD + invperms[i, 3][d]
            parts.append(_wrap_idx(arrA.reshape(-1, 128).ravel()))
            parts.append(_wrap_idx(arrB.reshape(-1, 128).ravel()))
        idxc.append(np.concatenate(parts, axis=1))
    idxc = np.stack(idxc)  # [NC, 128, COLS_C]

    return dis, perms, calls, K, idxg, idxc, groups


# ---------------- bass kernel builders ----------------

def _bass_mods():
    import sys
    if "/opt/trn_rl_repo" not in sys.path:
        sys.path.insert(0, "/opt/trn_rl_repo")
    import concourse.bass as bass
    import concourse.bacc as bacc
    import concourse.tile as tile
    from concourse import mybir
    from concourse.bass_utils import run_bass_kernel_spmd
    return bass, bacc, tile, mybir, run_bass_kernel_spmd


def _build_mm(NQ=1):
    """h1p = (x @ W1) * dis for own 12544-node shard. xT input pre-transposed."""
    bass, bacc, tile, mybir, _ = _bass_mods()
    from contextlib import ExitStack
    nc = bacc.Bacc()
    xT = nc.declare_dram_parameter("xT", [E_CH, PC_PAD], mybir.dt.float32, isOutput=False)
    W1 = nc.declare_dram_parameter("W1", [E_CH, HID], mybir.dt.float32, isOutput=False)
    disp = nc.declare_dram_parameter("disp", [128, NTILES], mybir.dt.float32, isOutput=False)
    out = nc.declare_dram_parameter("out", [PC_PAD, HID], mybir.dt.float32, isOutput=True)
    G = 7
    with tile.TileContext(nc) as tc, ExitStack() as ctx:
        wp = ctx.enter_context(tc.tile_pool(name="wp", bufs=1))
        sb = ctx.enter_context(tc.tile_pool(name="sb", bufs=3))
        ps = ctx.enter_context(tc.tile_pool(name="ps", bufs=4, space="PSUM"))
        w1 = wp.tile([E_CH, HID], mybir.dt.float32, tag="w1")
        nc.sync.dma_start(out=w1[:], in_=W1[:, :])
        dis_sb = wp.tile([128, NTILES], mybir.dt.float32, tag="dis")
        nc.sync.dma_start(out=dis_sb[:], in_=disp[:, :])
        for g in range(0, NTILES, G):
            ng = min(G, NTILES - g)
            xt = sb.tile([E_CH, G * 128], mybir.dt.float32, tag="xt")
            nc.sync.dma_start(out=xt[:, :ng * 128],
                              in_=xT[:, g * 128:(g + ng) * 128])
            ot = sb.tile([128, G * HID], mybir.dt.float32, tag="ot")
            for j in range(ng):
                pt = ps.tile([128, HID], mybir.dt.float32, space="PSUM", tag="pt")
                nc.tensor.matmul(pt[:], lhsT=xt[:, j * 128:(j + 1) * 128],
                                 rhs=w1[:], start=True, stop=True)
                nc.vector.tensor_scalar_mul(ot[:, j * HID:(j + 1) * HID], pt[:],
                                            dis_sb[:, g + j:g + j + 1])
            nc.sync.dma_start(
                out=out[g * 128:(g + ng) * 128, :].rearrange(
                    "(n p) f -> p n f", p=128),
                in_=ot[:, :ng * HID].rearrange("p (n f) -> p n f", n=ng))
    nc.compile()
    return nc


def _build_agg(calls, K, cols_g, cols_c, groups, layer):
    """Aggregation launch. layer=1: epilogue relu(dis*(dis*S+b1)) -> [PC_PAD, 64].
    layer=2: epilogue (S*dis)@W2 + b2 -> [PC_PAD, 2]."""
    bass, bacc, tile, mybir, _ = _bass_mods()
    from contextlib import ExitStack
    from concourse.masks import make_identity
    NQ = 4
    nc = bacc.Bacc(num_swdge_queues=NQ, dynamic_dma_scratch_size=16384 * NQ)
    tabs = [nc.declare_dram_parameter(f"tab{c}", [CHUNK_ROWS, HID], mybir.dt.float32,
                                      isOutput=False) for c in range(NCHUNK)]
    idxg = nc.declare_dram_parameter("idxg", [128, cols_g], mybir.dt.int16, isOutput=False)
    idxc = nc.declare_dram_parameter("idxc", [128, cols_c], mybir.dt.int16, isOutput=False)
    disp = nc.declare_dram_parameter("disp", [128, NTILES], mybir.dt.float32, isOutput=False)
    if layer == 1:
        b1b = nc.declare_dram_parameter("b1b", [128, HID], mybir.dt.float32, isOutput=False)
        out = nc.declare_dram_parameter("out", [PC_PAD, HID], mybir.dt.float32, isOutput=True)
    else:
        W2 = nc.declare_dram_parameter("W2", [HID, OUT], mybir.dt.float32, isOutput=False)
        b2b = nc.declare_dram_parameter("b2b", [128, OUT], mybir.dt.float32, isOutput=False)
        out = nc.declare_dram_parameter("out", [PC_PAD, OUT], mybir.dt.float32, isOutput=True)
    acc01 = nc.dram_tensor("acc01", [2 * PC_PAD, HID], mybir.dt.float32)
    acc23 = nc.dram_tensor("acc23", [2 * PC_PAD, HID], mybir.dt.float32)
    accs = [acc01, acc01, acc23, acc23]
    accoff = [0, PC_PAD, 0, PC_PAD]

    with tile.TileContext(nc) as tc, ExitStack() as ctx:
        cst = ctx.enter_context(tc.tile_pool(name="cst", bufs=1))
        ib = ctx.enter_context(tc.tile_pool(name="ib", bufs=2))
        stp = ctx.enter_context(tc.tile_pool(name="stp", bufs=4))
        csp = ctx.enter_context(tc.tile_pool(name="csp", bufs=3))
        ab = ctx.enter_context(tc.tile_pool(name="ab", bufs=2))
        ep = ctx.enter_context(tc.tile_pool(name="ep", bufs=2))
        ps = ctx.enter_context(tc.tile_pool(name="ps", bufs=4, space="PSUM"))

        dis_sb = cst.tile([128, NTILES], mybir.dt.float32, tag="dis")
        nc.sync.dma_start(out=dis_sb[:], in_=disp[:, :])
        if layer == 1:
            b1t = cst.tile([128, HID], mybir.dt.float32, tag="b1t")
            nc.sync.dma_start(out=b1t[:], in_=b1b[:, :])
        else:
            w2t = cst.tile([HID, OUT], mybir.dt.float32, tag="w2t")
            nc.sync.dma_start(out=w2t[:], in_=W2[:, :])
            b2t = cst.tile([128, OUT], mybir.dt.float32, tag="b2t")
            nc.sync.dma_start(out=b2t[:], in_=b2b[:, :])
            ident = cst.tile([128, 128], mybir.dt.float32, tag="ident")
            make_identity(nc, ident[:])

        # ---- chunk passes ----
        sall = cst.tile([128, NTILES * HID], mybir.dt.float32, tag="sall")
        icsb = cst.tile([128, cols_c], mybir.dt.int16, tag="icsb")
        nc.sync.dma_start(out=icsb[:], in_=idxc[:, :])
        # per-group idxc column offsets
        gco = []
        _co = 0
        for grp in groups:
            gco.append(_co)
            _co += 2 * (2 * len(grp) * 128 // 16)
        a_emitted = False
        qn = 0
        goff = 0  # column offset into idxg (int16 cols = positions/16)
        # per-chunk idx SBUF tiles loaded lazily per pass
        cur_chunk = -1
        idx_sb = None
        chunk_goff = 0
        # precompute per-chunk column extents
        chunk_cols = {c: sum(tot for (cc, _, tot) in calls if cc == c) for c in range(NCHUNK)}
        for (c, tiles, tot) in calls:
            if c == 3 and not a_emitted:
                a_emitted = True
                for gi, grp in enumerate(groups):
                    ng = len(grp)
                    niA = 2 * ng * 128
                    ccols = niA // 16
                    stA = csp.tile([128, 2 * 16 * HID], mybir.dt.float32, tag="cstage")
                    nc.gpsimd.dma_gather(
                        out_ap=stA[:, :2 * ng * HID].rearrange(
                            "p (k f) -> p k f", k=2 * ng),
                        in_ap=acc01[:, :],
                        idxs_ap=icsb[:, gco[gi]:gco[gi] + ccols],
                        num_idxs=niA, num_idxs_reg=niA, elem_size=HID,
                        queue_num=qn, single_packet=False)
                    qn = (qn + 1) % NQ
                    g0 = grp[0]
                    nc.vector.tensor_reduce(
                        out=sall[:, g0 * HID:(g0 + ng) * HID],
                        in_=bass.AP(stA.tensor, stA[:].offset,
                                    [stA[:].ap[0], [2 * HID, ng],
                                     [1, HID], [HID, 2]]),
                        axis=mybir.AxisListType.X, op=mybir.AluOpType.add)
            if c != cur_chunk:
                cur_chunk = c
                chunk_goff = goff
                ccols = chunk_cols[c] * 8  # int16 cols per pass (= positions/16)
                idx_sb = ib.tile([128, ccols], mybir.dt.int16, tag="idx")
                nc.sync.dma_start(out=idx_sb[:], in_=idxg[:, goff:goff + ccols])
            ni = tot * 128
            stage = stp.tile([128, tot * HID], mybir.dt.float32, tag="stage")
            lo = (goff - chunk_goff)
            nc.gpsimd.dma_gather(
                out_ap=stage[:].rearrange("p (k f) -> p k f", k=tot),
                in_ap=tabs[c][:, :],
                idxs_ap=idx_sb[:, lo:lo + tot * 8],
                num_idxs=ni,
                num_idxs_reg=ni,
                elem_size=HID,
                queue_num=qn,
                single_packet=False,
            )
            qn = (qn + 1) % NQ
            ntc = len(tiles)
            astrip = ab.tile([128, 12 * HID], mybir.dt.float32, tag="astrip")
            for j, (t, kt, off) in enumerate(tiles):
                nc.vector.tensor_reduce(
                    out=astrip[:, j * HID:(j + 1) * HID],
                    in_=bass.AP(stage.tensor, stage[:].offset + off * HID,
                                [stage[:].ap[0], [1, HID], [HID, kt]]),
                    axis=mybir.AxisListType.X,
                    op=mybir.AluOpType.add,
                )
            t0 = tiles[0][0]
            nc.sync.dma_start(
                out=accs[c][accoff[c] + t0 * 128:accoff[c] + (t0 + ntc) * 128,
                            :].rearrange("(n p) f -> p n f", p=128),
                in_=astrip[:, :ntc * HID].rearrange("p (n f) -> p n f", n=ntc))
            goff += tot * 8

        # ---- combine phase B + epilogue ----
        for gi, grp in enumerate(groups):
            ng = len(grp)
            niA = 2 * ng * 128
            ccols = niA // 16
            stB = csp.tile([128, 2 * 16 * HID], mybir.dt.float32, tag="cstage")
            nc.gpsimd.dma_gather(
                out_ap=stB[:, :2 * ng * HID].rearrange("p (k f) -> p k f", k=2 * ng),
                in_ap=acc23[:, :],
                idxs_ap=icsb[:, gco[gi] + ccols:gco[gi] + 2 * ccols],
                num_idxs=niA, num_idxs_reg=niA, elem_size=HID,
                queue_num=qn, single_packet=False)
            qn = (qn + 1) % NQ
            FO = HID if layer == 1 else OUT
            ostrip = ab.tile([128, 16 * FO], mybir.dt.float32, tag="ostrip")
            sB = ep.tile([128, 16 * HID], mybir.dt.float32, tag="sB")
            nc.vector.tensor_reduce(
                out=sB[:, :ng * HID],
                in_=bass.AP(stB.tensor, stB[:].offset,
                            [stB[:].ap[0], [2 * HID, ng], [1, HID], [HID, 2]]),
                axis=mybir.AxisListType.X, op=mybir.AluOpType.add)
            g0 = grp[0]
            nc.vector.tensor_add(sB[:, :ng * HID],
                                 sall[:, g0 * HID:(g0 + ng) * HID],
                                 sB[:, :ng * HID])
            dview = bass.AP(dis_sb.tensor, dis_sb[:].offset + g0,
                            [dis_sb[:].ap[0], [1, ng], [0, HID]])
            ssv = bass.AP(sB.tensor, sB[:].offset,
                          [sB[:].ap[0], [HID, ng], [1, HID]])
            nc.vector.tensor_tensor(out=ssv, in0=ssv, in1=dview,
                                    op=mybir.AluOpType.mult)
            if layer == 1:
                b1view = bass.AP(b1t.tensor, b1t[:].offset,
                                 [b1t[:].ap[0], [0, ng], [1, HID]])
                nc.vector.tensor_tensor(out=ssv, in0=ssv, in1=b1view,
                                        op=mybir.AluOpType.add)
                nc.vector.tensor_tensor(out=ssv, in0=ssv, in1=dview,
                                        op=mybir.AluOpType.mult)
                nc.vector.tensor_scalar_max(ostrip[:, :ng * HID], sB[:, :ng * HID], 0.0)
            else:
                for j, t in enumerate(grp):
                    put = ps.tile([HID, 128], mybir.dt.float32, space="PSUM", tag="put")
                    nc.tensor.transpose(out=put[:],
                                        in_=sB[:, j * HID:(j + 1) * HID],
                                        identity=ident[:])
                    ut = ep.tile([HID, 128], mybir.dt.float32, tag="ut")
                    nc.vector.tensor_copy(out=ut[:], in_=put[:])
                    po = ps.tile([128, OUT], mybir.dt.float32, space="PSUM", tag="po")
                    nc.tensor.matmul(po[:], lhsT=ut[:], rhs=w2t[:], start=True, stop=True)
                    nc.vector.tensor_add(
                        ostrip[:, j * OUT:(j + 1) * OUT], po[:], b2t[:])
            nc.sync.dma_start(
                out=out[g0 * 128:(g0 + ng) * 128, :].rearrange(
                    "(n p) f -> p n f", p=128),
                in_=ostrip[:, :ng * FO].rearrange("p (n f) -> p n f", n=ng))
    nc.compile()
    return nc


def _mk_tables(h):
    """h [N, 64] f32 -> 4 chunk tensors [25001, 64] with zero row at local 25000."""
    tabs = []
    for c in range(NCHUNK):
        t = np.zeros((CHUNK_ROWS, HID), np.float32)
        t[:CHUNK] = h[c * CHUNK:(c + 1) * CHUNK]
        tabs.append(t)
    return tabs


def kernel(x, edge_index, W1, b1, W2, b2):
    x = np.asarray(x, dtype=np.float32)
    W1 = np.asarray(W1, dtype=np.float32)
    b1 = np.asarray(b1, dtype=np.float32)
    W2 = np.asarray(W2, dtype=np.float32)
    b2 = np.asarray(b2, dtype=np.float32)

    bass, bacc, tile, mybir, run_spmd = _bass_mods()

    dis, perms, calls, K, idxg, idxc, groups = _host_prep(edge_index)
    cols_g = idxg.shape[2]
    cols_c = idxc.shape[2]

    cores = list(range(NC))

    # ---- launch 1: h1p = (x @ W1) * dis ----
    nc1 = _build_mm()
    def _disp(i):
        dp = np.concatenate([dis[i * PERCORE:(i + 1) * PERCORE],
                             np.ones(PC_PAD - PERCORE, np.float32)])
        return np.ascontiguousarray(dp.reshape(NTILES, 128).T)

    in1 = []
    for i in cores:
        xT = np.zeros((E_CH, PC_PAD), np.float32)
        xT[:, :PERCORE] = x[i * PERCORE:(i + 1) * PERCORE].T
        in1.append({"xT": xT, "W1": W1, "disp": _disp(i)})
    r1 = run_spmd(nc1, in1, core_ids=cores)
    h1p = np.concatenate([np.asarray(r1.results[i]["out"])[:PERCORE] for i in cores])

    # ---- launch 2: layer-1 aggregation + activation -> a1p ----
    nc2 = _build_agg(calls, K, cols_g, cols_c, groups, layer=1)
    tabs1 = _mk_tables(h1p)
    b1bc = np.broadcast_to(b1, (128, HID)).copy()
    in2 = []
    for i in cores:
        m = {f"tab{c}": tabs1[c] for c in range(NCHUNK)}
        m.update({"idxg": idxg[i], "idxc": idxc[i], "disp": _disp(i), "b1b": b1bc})
        in2.append(m)
    r2 = run_spmd(nc2, in2, core_ids=cores)
    a1p = np.concatenate([np.asarray(r2.results[i]["out"])[:PERCORE] for i in cores])

    # ---- launch 3: layer-2 aggregation + head -> out ----
    nc3 = _build_agg(calls, K, cols_g, cols_c, groups, layer=2)
    tabs2 = _mk_tables(a1p)
    b2bc = np.broadcast_to(b2, (128, OUT)).copy()
    in3 = []
    for i in cores:
        m = {f"tab{c}": tabs2[c] for c in range(NCHUNK)}
        m.update({"idxg": idxg[i], "idxc": idxc[i], "disp": _disp(i),
                  "W2": W2, "b2b": b2bc})
        in3.append(m)
    r3 = run_spmd(nc3, in3, core_ids=cores)
    outv = np.concatenate([np.asarray(r3.results[i]["out"])[:PERCORE] for i in cores])
    return outv.astype(np.float32)

